# revision 1
# baseline (speedup 1.0000x reference)
"""Trainium2 Bass kernel for GCNN operator:
    h   = einsum('bnf,nfg->bng', x, kernel)   # per-node feature transform
    out = einsum('nm,bmg->bng', A, h) + bias  # dense adjacency aggregation

Sharding: node dim N row-sharded across 8 cores. Each core computes h for
its 2048 nodes (DVE), AllGathers h (small), then computes its row-shard of
A @ H on the TensorEngine while streaming its A-shard (pre-transposed and
cast to fp16 on host) from HBM at full DMA bandwidth.

Self-contained: hardcodes shapes; only imports concourse + numpy/jax.
"""

import numpy as np

B, N, F, G = 2, 16384, 16, 16
NCORES = 8
P = 128                    # SBUF partitions
C = B * G                  # 32 fused (batch, out-feature) columns
NT = 512                   # matmul moving-operand free-dim per instruction


def build_nc(n=N, ncores=NCORES, at_bufs=6, km=4):
    """Build the per-core Bass program (SPMD: same program on all cores)."""
    import concourse.bass as bass
    import concourse.mybir as mybir
    import concourse.tile as tile
    from concourse import bacc
    from concourse.masks import make_identity

    f32 = mybir.dt.float32
    f16 = mybir.dt.float16

    nl = n // ncores           # nodes per core
    j_n = nl // P              # local node blocks (16 at full size)
    mj = n // P                # contraction blocks (128 at full size)
    nt_n = max(nl // NT, 1)    # rhs free-dim chunks per at-tile
    ntc = min(NT, nl)
    km = min(km, mj)           # contraction blocks batched per DMA

    nc = bacc.Bacc(
        "TRN2", target_bir_lowering=False, debug=False, num_devices=ncores
    )

    at = nc.dram_tensor("at", [n, nl], f16, kind="ExternalInput")
    xs = nc.dram_tensor("xs", [B, nl, F], f16, kind="ExternalInput")
    ks = nc.dram_tensor("ks", [nl, F, G], f16, kind="ExternalInput")
    bs = nc.dram_tensor("bs", [nl, G], f32, kind="ExternalInput")
    outs = nc.dram_tensor("outs", [P, j_n * C], f32, kind="ExternalOutput")

    with tile.TileContext(nc) as tc:
        with (
            tc.tile_pool(name="dram", bufs=1, space="DRAM") as dram,
            tc.tile_pool(name="const", bufs=1) as const,
            tc.tile_pool(name="work", bufs=2) as work,
            tc.tile_pool(name="atp", bufs=at_bufs) as atp,
            tc.tile_pool(name="pacc", bufs=1, space="PSUM") as pacc,
            tc.tile_pool(name="ptr", bufs=2, space="PSUM") as ptr,
        ):
            # h bounce/gather buffers are partition-major: [P, j_n*C] per rank,
            # AllGather concats ranks on axis 0 -> [ncores*P, j_n*C]. Global
            # m-block r*j_n + j lands at rows [r*P:(r+1)*P], cols [j*C:(j+1)*C],
            # i.e. hq free-dim order (r, j, c) == m-block-major, matching the
            # matmul's hq[:, m*C:(m+1)*C] slicing.
            w_h = j_n * C
            h_bounce = dram.tile([P, w_h], f16)
            h_full = dram.tile([ncores * P, w_h], f16, addr_space="Shared")

            # ---- prologue loads (SWDGE queue, separate from the A stream) ----
            x_sb = const.tile([P, j_n, B, F], f16)
            for b in range(B):
                nc.gpsimd.dma_start(
                    out=x_sb[:, :, b, :],
                    in_=xs[b].rearrange("(j p) f -> p j f", p=P),
                )
            k_sb = const.tile([P, j_n, F, G], f16)
            nc.gpsimd.dma_start(
                out=k_sb[:, :, :, :],
                in_=ks.ap().rearrange("(j p) f g -> p j f g", p=P),
            )
            bias2 = const.tile([P, j_n, C], f32)
            for b in range(B):
                nc.gpsimd.dma_start(
                    out=bias2[:, :, b * G : (b + 1) * G],
                    in_=bs.ap().rearrange("(j p) g -> p j g", p=P),
                )
            identity = const.tile([C, C], f32)
            make_identity(nc, identity[:, :])

            # ---- h = einsum('bnf,nfg->bng') on DVE, local nodes ----
            h_f32 = const.tile([P, j_n, B, G], f32)
            for b in range(B):
                prod = work.tile([P, j_n, G, F], f32, tag="prod")
                nc.vector.tensor_tensor(
                    prod[:, :, :, :],
                    x_sb[:, :, b, None, :].to_broadcast([P, j_n, G, F]),
                    k_sb[:, :, :, :].rearrange("p j f g -> p j g f"),
                    mybir.AluOpType.mult,
                )
                nc.vector.tensor_reduce(
                    h_f32[:, :, b, :],
                    prod[:, :, :, :],
                    axis=mybir.AxisListType.X,
                    op=mybir.AluOpType.add,
                )
            h16 = const.tile([P, j_n, B, G], f16)
            nc.vector.tensor_copy(h16[:, :, :, :], h_f32[:, :, :, :])
            nc.gpsimd.dma_start(
                out=h_bounce[:, :],
                in_=h16[:, :, :, :].rearrange("p j b g -> p (j b g)"),
            )

            # ---- AllGather h -> full H [n, C] on every core ----
            nc.gpsimd.collective_compute(
                "AllGather",
                mybir.AluOpType.bypass,
                replica_groups=[list(range(ncores))],
                ins=[h_bounce[:, :].opt()],
                outs=[h_full[:, :].opt()],
            )
            hq = const.tile([P, mj * C], f16)
            nc.gpsimd.dma_start(
                out=hq[:, :].rearrange("p (r w) -> p r w", w=w_h),
                in_=h_full.rearrange("(r p) w -> p r w", p=P),
            )

            # ---- main loop: out^T[c, n_local] += H_m^T-block @ A^T tile ----
            acc = [
                pacc.tile([C, ntc], f32, tag=f"acc{t}", name=f"acc{t}")
                for t in range(nt_n)
            ]
            # A^T stream: km contraction blocks per DMA (amortizes per-DMA
            # fixed cost), alternating between the two HWDGE rings (SP/ACT).
            at_r = at.ap().rearrange("(mb km p) nl -> mb p km nl", p=P, km=km)
            for mb in range(mj // km):
                eng = nc.scalar if mb % 2 else nc.sync
                at_t = atp.tile([P, km, nl], f16, tag="at_t", name="at_t")
                eng.dma_start(out=at_t[:, :, :], in_=at_r[mb])
                for kk in range(km):
                    m = mb * km + kk
                    for t in range(nt_n):
                        nc.tensor.matmul(
                            acc[t][:, :],
                            hq[:, m * C : (m + 1) * C],
                            at_t[:, kk, t * ntc : (t + 1) * ntc],
                            start=(m == 0),
                            stop=(m == mj - 1),
                        )

            # ---- epilogue: transpose out^T back to [node, c], add bias ----
            outT = work.tile([C, nl], f32, tag="outT")
            for t in range(nt_n):
                nc.vector.tensor_copy(outT[:, t * ntc : (t + 1) * ntc], acc[t][:, :])
            out_sb = work.tile([P, j_n, C], f32, tag="out_sb")
            for j in range(j_n):
                pt = ptr.tile([P, C], f32, tag="pt", name="pt")
                nc.tensor.transpose(
                    pt[:, :], outT[:, j * P : (j + 1) * P], identity[:, :]
                )
                nc.vector.tensor_add(out_sb[:, j, :], pt[:, :], bias2[:, j, :])
            nc.sync.dma_start(
                out=outs.ap(), in_=out_sb[:, :, :].rearrange("p j c -> p (j c)")
            )

    nc.compile()
    return nc


_NC_CACHE = {}


def _get_nc(n=N, ncores=NCORES):
    key = (n, ncores)
    if key not in _NC_CACHE:
        _NC_CACHE[key] = build_nc(n, ncores)
    return _NC_CACHE[key]


def make_in_maps(x, A, kern, bias, n=N, ncores=NCORES):
    nl = n // ncores
    in_maps = []
    for r in range(ncores):
        sl = slice(r * nl, (r + 1) * nl)
        in_maps.append(
            {
                "at": A[sl, :].T.astype(np.float16),
                "xs": np.ascontiguousarray(x[:, sl, :]).astype(np.float16),
                "ks": np.ascontiguousarray(kern[sl]).astype(np.float16),
                "bs": np.ascontiguousarray(bias[sl]),
            }
        )
    return in_maps


def assemble_out(results, n=N, ncores=NCORES):
    nl = n // ncores
    j_n = nl // P
    parts = []
    for r in range(ncores):
        o = results[r]["outs"].reshape(P, j_n, B, G)
        parts.append(o.transpose(2, 1, 0, 3).reshape(B, nl, G))
    return np.ascontiguousarray(np.concatenate(parts, axis=1))


def run(inputs, n=N, ncores=NCORES, trace=False, **spmd_kwargs):
    from concourse.bass_utils import run_bass_kernel_spmd

    x = np.asarray(inputs["x"], dtype=np.float32)
    A = np.asarray(inputs["A"], dtype=np.float32)
    kern = np.asarray(inputs["kernel"], dtype=np.float32)
    bias = np.asarray(inputs["bias"], dtype=np.float32)
    nc = _get_nc(n, ncores)
    in_maps = make_in_maps(x, A, kern, bias, n, ncores)
    res = run_bass_kernel_spmd(
        nc, in_maps, list(range(ncores)), trace=trace, **spmd_kwargs
    )
    out = assemble_out(res.results, n, ncores)
    return out, res


def kernel(**inputs) -> np.ndarray:
    out, _ = run(inputs)
    return out



# revision 2
# speedup vs baseline: 1.3444x; 1.3444x over previous
"""Trainium2 Bass kernel for GCNN operator:
    h   = einsum('bnf,nfg->bng', x, kernel)   # per-node feature transform
    out = einsum('nm,bmg->bng', A, h) + bias  # dense adjacency aggregation

Sharding: node dim N row-sharded across 8 cores for the A@h matmul only.
Every core redundantly computes the FULL h on its DVE (x and kernel are
small), so there is NO collective at all — no cross-core barrier, no skew
sensitivity. The A-shard (pre-transposed, pre-tiled and cast to fp16 on
host so every DMA descriptor is one 16KB contiguous run) streams from HBM
while the TensorEngine accumulates out^T = sum_m H_m^T @ A^T_m. Bias is
added during the PSUM->SBUF drain; the host undoes the out^T layout.

Self-contained: hardcodes shapes; only imports concourse + numpy.
"""

import numpy as np

B, N, F, G = 2, 16384, 16, 16
NCORES = 8
P = 128                    # SBUF partitions
C = B * G                  # 32 fused (batch, out-feature) columns
NT = 512                   # matmul moving-operand free-dim per instruction
KM = 4                     # contraction j-blocks per A-stream DMA (2MB tiles)
JJ = 16                    # j-blocks per H compute chunk
AT_BUFS = 5                # A-stream double buffering depth


def build_nc(n=N, ncores=NCORES, at_bufs=AT_BUFS):
    """Build the per-core Bass program (SPMD: same program on all cores)."""
    import concourse.bass as bass
    import concourse.mybir as mybir
    import concourse.tile as tile
    from concourse import bacc

    f32 = mybir.dt.float32
    f16 = mybir.dt.float16

    nl = n // ncores           # local output rows per core
    jn = n // P                # contraction j-blocks over FULL n
    km = min(KM, jn)           # j-blocks per A DMA
    mb_n = jn // km            # A-stream DMA count
    jj = min(JJ, jn)           # j-blocks per H chunk
    nch = jn // jj             # H chunks
    ntc = min(NT, nl)          # matmul moving free-dim
    nt_n = max(nl // ntc, 1)   # acc tiles

    nc = bacc.Bacc("TRN2", target_bir_lowering=False, debug=False, num_devices=1)

    at = nc.dram_tensor("at", [mb_n, P, km * nl], f16, kind="ExternalInput")
    xq = nc.dram_tensor("xq", [P, jn * B * F], f16, kind="ExternalInput")
    kq = nc.dram_tensor("kq", [P, jn * G * F], f16, kind="ExternalInput")
    bsT = nc.dram_tensor("bsT", [C, nl], f32, kind="ExternalInput")
    outs = nc.dram_tensor("outs", [C, nl], f32, kind="ExternalOutput")

    with tile.TileContext(nc) as tc:
        with (
            tc.tile_pool(name="const", bufs=1) as const,
            tc.tile_pool(name="work", bufs=2) as work,
            tc.tile_pool(name="atp", bufs=at_bufs) as atp,
            tc.tile_pool(name="pacc", bufs=1, space="PSUM") as pacc,
        ):
            # ---- prologue loads (SWDGE queue, separate from the A stream) ----
            xs = const.tile([P, jn, B, F], f16)
            nc.gpsimd.dma_start(
                out=xs[:, :, :, :],
                in_=xq.ap().rearrange("p (j b f) -> p j b f", b=B, f=F),
            )
            ksq = []
            for q in range(nch):
                kt = const.tile([P, jj, G, F], f16, tag=f"ks{q}", name=f"ks{q}")
                nc.gpsimd.dma_start(
                    out=kt[:, :, :, :],
                    in_=kq.ap().rearrange("p (j g f) -> p j g f", g=G, f=F)[
                        :, q * jj : (q + 1) * jj
                    ],
                )
                ksq.append(kt)
            biasT = const.tile([C, nl], f32)
            nc.gpsimd.dma_start(out=biasT[:, :], in_=bsT.ap())

            # ---- full H on DVE, chunked: hq_q[p, j, (b g)] = sum_f x*k ----
            hqs = []
            with nc.allow_low_precision(reason="h accum over F=16 in fp16"):
                for q in range(nch):
                    hq = const.tile([P, jj, C], f16, tag=f"hq{q}", name=f"hq{q}")
                    for b in range(B):
                        prod = work.tile([P, jj, G, F], f16, tag="prod")
                        nc.vector.tensor_tensor(
                            prod[:, :, :, :],
                            xs[:, q * jj : (q + 1) * jj, b, None, :].to_broadcast(
                                [P, jj, G, F]
                            ),
                            ksq[q][:, :, :, :],
                            mybir.AluOpType.mult,
                        )
                        nc.vector.tensor_reduce(
                            hq[:, :, b * G : (b + 1) * G],
                            prod[:, :, :, :],
                            axis=mybir.AxisListType.X,
                            op=mybir.AluOpType.add,
                        )
                    hqs.append(hq)

            # ---- main loop: out^T[c, nl] += H_m^T-block @ A^T tile ----
            acc = [
                pacc.tile([C, ntc], f32, tag=f"acc{t}", name=f"acc{t}")
                for t in range(nt_n)
            ]
            for mb in range(mb_n):
                eng = nc.scalar if mb % 2 else nc.sync
                at_t = atp.tile([P, km, nl], f16, tag="at_t", name="at_t")
                eng.dma_start(
                    out=at_t[:, :, :],
                    in_=at.ap()[mb].rearrange("p (km nl) -> p km nl", km=km),
                )
                for kk in range(km):
                    m = mb * km + kk
                    q, jl = divmod(m, jj)
                    for t in range(nt_n):
                        nc.tensor.matmul(
                            acc[t][:, :],
                            hqs[q][:, jl, :],
                            at_t[:, kk, t * ntc : (t + 1) * ntc],
                            start=(m == 0),
                            stop=(m == jn - 1),
                        )

            # ---- epilogue: drain PSUM with fused bias add, store out^T ----
            outT = work.tile([C, nl], f32, tag="outT")
            for t in range(nt_n):
                nc.vector.tensor_add(
                    outT[:, t * ntc : (t + 1) * ntc],
                    acc[t][:, :],
                    biasT[:, t * ntc : (t + 1) * ntc],
                )
            nc.sync.dma_start(out=outs.ap(), in_=outT[:, :])

    nc.compile()
    return nc


_NC_CACHE = {}


def _get_nc(n=N, ncores=NCORES):
    key = (n, ncores)
    if key not in _NC_CACHE:
        _NC_CACHE[key] = build_nc(n, ncores)
    return _NC_CACHE[key]


def make_in_maps(x, A, kern, bias, n=N, ncores=NCORES):
    nl = n // ncores
    jn = n // P
    km = min(KM, jn)
    mb_n = jn // km

    # Shared across cores: x and kernel in [p-major] DVE-friendly layouts.
    # xq[p, j, b, f] = x[b, j*P+p, f];  kq[p, j, g, f] = kern[j*P+p, f, g]
    x16 = x.astype(np.float16).transpose(1, 0, 2).reshape(jn, P, B, F)
    xq = np.ascontiguousarray(x16.transpose(1, 0, 2, 3)).reshape(P, jn * B * F)
    k16 = kern.astype(np.float16).transpose(0, 2, 1).reshape(jn, P, G, F)
    kq = np.ascontiguousarray(k16.transpose(1, 0, 2, 3)).reshape(P, jn * G * F)

    A16 = A.astype(np.float16)
    in_maps = []
    for r in range(ncores):
        sl = slice(r * nl, (r + 1) * nl)
        # at[mb, p, kk*nl:] = A^T[(mb*km+kk)*P + p, r-shard] = A[shard, row].T
        at = np.ascontiguousarray(
            A16[sl, :].T.reshape(mb_n, km, P, nl).transpose(0, 2, 1, 3)
        ).reshape(mb_n, P, km * nl)
        # bsT[(b g), nl] = bias[shard][nl, g] for both b
        bT = np.ascontiguousarray(bias[sl].T)  # [G, nl]
        bsT = np.ascontiguousarray(np.tile(bT, (B, 1)))  # [C, nl]
        in_maps.append({"at": at, "xq": xq, "kq": kq, "bsT": bsT})
    return in_maps


def assemble_out(results, n=N, ncores=NCORES):
    nl = n // ncores
    parts = []
    for r in range(ncores):
        o = results[r]["outs"].reshape(B, G, nl)
        parts.append(o.transpose(0, 2, 1))  # [B, nl, G]
    return np.ascontiguousarray(np.concatenate(parts, axis=1))


def run(inputs, n=N, ncores=NCORES, trace=False, **spmd_kwargs):
    from concourse.bass_utils import run_bass_kernel_spmd

    x = np.asarray(inputs["x"], dtype=np.float32)
    A = np.asarray(inputs["A"], dtype=np.float32)
    kern = np.asarray(inputs["kernel"], dtype=np.float32)
    bias = np.asarray(inputs["bias"], dtype=np.float32)
    nc = _get_nc(n, ncores)
    in_maps = make_in_maps(x, A, kern, bias, n, ncores)
    res = run_bass_kernel_spmd(
        nc, in_maps, list(range(ncores)), trace=trace, **spmd_kwargs
    )
    out = assemble_out(res.results, n, ncores)
    return out, res


def kernel(**inputs) -> np.ndarray:
    out, _ = run(inputs)
    return out


# revision 6
# speedup vs baseline: 1.5635x; 1.1630x over previous
"""Trainium2 Bass kernel for GCNN operator:
    h   = einsum('bnf,nfg->bng', x, kernel)   # per-node feature transform
    out = einsum('nm,bmg->bng', A, h) + bias  # dense adjacency aggregation

Sharding: node dim N row-sharded across 8 cores for the A@h matmul only.
Every core redundantly computes the FULL h on its DVE (x and kernel are
small), so there is NO collective at all — no cross-core barrier, no skew
sensitivity. The A-shard (pre-transposed, pre-tiled and cast to fp16 on
host so every DMA descriptor is one 16KB contiguous run) streams from HBM
while the TensorEngine accumulates out^T = sum_m H_m^T @ A^T_m. Bias is
added during the PSUM->SBUF drain; the host undoes the out^T layout.

Self-contained: hardcodes shapes; only imports concourse + numpy.
"""

import numpy as np

B, N, F, G = 2, 16384, 16, 16
NCORES = 8
P = 128                    # SBUF partitions
C = B * G                  # 32 fused (batch, out-feature) columns
NT = 512                   # matmul moving-operand free-dim per instruction
KM = 2                     # contraction j-blocks per A-stream DMA (1MB tiles)
JJ = 16                    # j-blocks per H compute chunk
AT_BUFS = 10               # A-stream double buffering depth


def build_nc(n=N, ncores=NCORES, at_bufs=AT_BUFS):
    """Build the per-core Bass program (SPMD: same program on all cores)."""
    import concourse.bass as bass
    import concourse.mybir as mybir
    import concourse.tile as tile
    from concourse import bacc

    f32 = mybir.dt.float32
    f16 = mybir.dt.float16

    nl = n // ncores           # local output rows per core
    jn = n // P                # contraction j-blocks over FULL n
    km = min(KM, jn)           # j-blocks per A DMA
    mb_n = jn // km            # A-stream DMA count
    jj = min(JJ, jn)           # j-blocks per H chunk
    nch = jn // jj             # H chunks
    ntc = min(NT, nl)          # matmul moving free-dim
    nt_n = max(nl // ntc, 1)   # acc tiles

    nc = bacc.Bacc("TRN2", target_bir_lowering=False, debug=False, num_devices=1)

    at = nc.dram_tensor("at", [mb_n, P, km * nl], f16, kind="ExternalInput")
    xq = nc.dram_tensor("xq", [P, jn * B * F], f16, kind="ExternalInput")
    kq = nc.dram_tensor("kq", [P, jn * G * F], f16, kind="ExternalInput")
    bsT = nc.dram_tensor("bsT", [C, nl], f32, kind="ExternalInput")
    outs = nc.dram_tensor("outs", [C, nl], f32, kind="ExternalOutput")

    with tile.TileContext(nc) as tc:
        with (
            tc.tile_pool(name="const", bufs=1) as const,
            tc.tile_pool(name="work", bufs=2) as work,
            tc.tile_pool(name="atp", bufs=at_bufs) as atp,
            tc.tile_pool(name="pacc", bufs=1, space="PSUM") as pacc,
        ):
            # ---- prologue loads on the two HWDGE rings, ahead of the A
            # stream in each ring's FIFO (ks0/xs land ~12us in, so the DVE
            # can produce hq chunk 0 early and the PE never head-stalls) ----
            ksq = []
            for q in range(nch):
                kt = const.tile([P, jj, G, F], f16, tag=f"ks{q}", name=f"ks{q}")
                ksq.append(kt)
            xs = const.tile([P, jn, B, F], f16)
            biasT = const.tile([C, nl], f32)

            kq_r = kq.ap().rearrange("p (j g f) -> p j g f", g=G, f=F)
            nc.scalar.dma_start(
                out=ksq[0][:, :, :, :], in_=kq_r[:, 0:jj]
            )
            nc.sync.dma_start(
                out=xs[:, :, :, :],
                in_=xq.ap().rearrange("p (j b f) -> p j b f", b=B, f=F),
            )
            for q in range(1, nch):
                eng = nc.scalar if q % 2 == 0 else nc.sync
                eng.dma_start(
                    out=ksq[q][:, :, :, :], in_=kq_r[:, q * jj : (q + 1) * jj]
                )
            nc.sync.dma_start(out=biasT[:, :], in_=bsT.ap())

            # ---- full H on DVE, chunked: hq_q[p, j, (b g)] = sum_f x*k ----
            hqs = []
            with nc.allow_low_precision(reason="h accum over F=16 in fp16"):
                for q in range(nch):
                    hq = const.tile([P, jj, C], f16, tag=f"hq{q}", name=f"hq{q}")
                    for b in range(B):
                        prod = work.tile([P, jj, G, F], f16, tag="prod")
                        nc.vector.tensor_tensor(
                            prod[:, :, :, :],
                            xs[:, q * jj : (q + 1) * jj, b, None, :].to_broadcast(
                                [P, jj, G, F]
                            ),
                            ksq[q][:, :, :, :],
                            mybir.AluOpType.mult,
                        )
                        nc.vector.tensor_reduce(
                            hq[:, :, b * G : (b + 1) * G],
                            prod[:, :, :, :],
                            axis=mybir.AxisListType.X,
                            op=mybir.AluOpType.add,
                        )
                    hqs.append(hq)

            # ---- main loop: out^T[c, nl] += H_m^T-block @ A^T tile ----
            acc = [
                pacc.tile([C, ntc], f32, tag=f"acc{t}", name=f"acc{t}")
                for t in range(nt_n)
            ]
            outT = work.tile([C, nl], f32, tag="outT")
            for mb in range(mb_n):
                eng = nc.scalar if mb % 2 else nc.sync
                at_t = atp.tile([P, km, nl], f16, tag="at_t", name="at_t")
                eng.dma_start(
                    out=at_t[:, :, :],
                    in_=at.ap()[mb].rearrange("p (km nl) -> p km nl", km=km),
                )
                if mb < mb_n - 1:
                    for kk in range(km):
                        m = mb * km + kk
                        q, jl = divmod(m, jj)
                        for t in range(nt_n):
                            nc.tensor.matmul(
                                acc[t][:, :],
                                hqs[q][:, jl, :],
                                at_t[:, kk, t * ntc : (t + 1) * ntc],
                                start=(m == 0),
                                stop=False,
                            )
                else:
                    # Final tile: t-outer so each acc closes in turn; drain
                    # each to SBUF with a fused bias add and store it while
                    # the PE still works on the later t slices.
                    for t in range(nt_n):
                        for kk in range(km):
                            m = mb * km + kk
                            q, jl = divmod(m, jj)
                            nc.tensor.matmul(
                                acc[t][:, :],
                                hqs[q][:, jl, :],
                                at_t[:, kk, t * ntc : (t + 1) * ntc],
                                start=(m == 0),
                                stop=(kk == km - 1),
                            )
                        nc.vector.tensor_add(
                            outT[:, t * ntc : (t + 1) * ntc],
                            acc[t][:, :],
                            biasT[:, t * ntc : (t + 1) * ntc],
                        )
                        eng2 = nc.scalar if t % 2 else nc.sync
                        eng2.dma_start(
                            out=outs.ap()[:, t * ntc : (t + 1) * ntc],
                            in_=outT[:, t * ntc : (t + 1) * ntc],
                        )

    nc.compile()
    return nc


_NC_CACHE = {}


def _get_nc(n=N, ncores=NCORES):
    key = (n, ncores)
    if key not in _NC_CACHE:
        _NC_CACHE[key] = build_nc(n, ncores)
    return _NC_CACHE[key]


def make_in_maps(x, A, kern, bias, n=N, ncores=NCORES):
    nl = n // ncores
    jn = n // P
    km = min(KM, jn)
    mb_n = jn // km

    # Shared across cores: x and kernel in [p-major] DVE-friendly layouts.
    # xq[p, j, b, f] = x[b, j*P+p, f];  kq[p, j, g, f] = kern[j*P+p, f, g]
    x16 = x.astype(np.float16).transpose(1, 0, 2).reshape(jn, P, B, F)
    xq = np.ascontiguousarray(x16.transpose(1, 0, 2, 3)).reshape(P, jn * B * F)
    k16 = kern.astype(np.float16).transpose(0, 2, 1).reshape(jn, P, G, F)
    kq = np.ascontiguousarray(k16.transpose(1, 0, 2, 3)).reshape(P, jn * G * F)

    A16 = A.astype(np.float16)
    in_maps = []
    for r in range(ncores):
        sl = slice(r * nl, (r + 1) * nl)
        # at[mb, p, kk*nl:] = A^T[(mb*km+kk)*P + p, r-shard] = A[shard, row].T
        at = np.ascontiguousarray(
            A16[sl, :].T.reshape(mb_n, km, P, nl).transpose(0, 2, 1, 3)
        ).reshape(mb_n, P, km * nl)
        # bsT[(b g), nl] = bias[shard][nl, g] for both b
        bT = np.ascontiguousarray(bias[sl].T)  # [G, nl]
        bsT = np.ascontiguousarray(np.tile(bT, (B, 1)))  # [C, nl]
        in_maps.append({"at": at, "xq": xq, "kq": kq, "bsT": bsT})
    return in_maps


def assemble_out(results, n=N, ncores=NCORES):
    nl = n // ncores
    parts = []
    for r in range(ncores):
        o = results[r]["outs"].reshape(B, G, nl)
        parts.append(o.transpose(0, 2, 1))  # [B, nl, G]
    return np.ascontiguousarray(np.concatenate(parts, axis=1))


def run(inputs, n=N, ncores=NCORES, trace=False, **spmd_kwargs):
    from concourse.bass_utils import run_bass_kernel_spmd

    x = np.asarray(inputs["x"], dtype=np.float32)
    A = np.asarray(inputs["A"], dtype=np.float32)
    kern = np.asarray(inputs["kernel"], dtype=np.float32)
    bias = np.asarray(inputs["bias"], dtype=np.float32)
    nc = _get_nc(n, ncores)
    in_maps = make_in_maps(x, A, kern, bias, n, ncores)
    res = run_bass_kernel_spmd(
        nc, in_maps, list(range(ncores)), trace=trace, **spmd_kwargs
    )
    out = assemble_out(res.results, n, ncores)
    return out, res


def kernel(**inputs) -> np.ndarray:
    out, _ = run(inputs)
    return out
